# revision 6
# baseline (speedup 1.0000x reference)
"""Trainium2 Bass kernel for nn_GaussianPerslayPhi (Gaussian persistence image).

out[n, p, i, j] = exp(-((d0-X_j)^2 + (d1-Y_i)^2) / (2 v^2)) / (2 pi v^2)
with d0 = diagrams[n,p,0], d1 = diagrams[n,p,1] - diagrams[n,p,0],
X_j = Y_j = -3 + (6/64)*j, output shape (64, 128, 64, 64, 1) fp32.

Key structure: the Gaussian separates into gx[n,p,j] * gy[n,p,i], two tiny
(128,512) factor tables per core.  Each core (8 total, data-parallel over n)
computes the factors with ScalarE exp, then expands them with one broadcast
tensor_tensor multiply per diagram ([128, 4096] tile, step-0 access patterns)
and streams the 2 MiB tiles to HBM.  The kernel is output-write bound:
16 MiB/core at ~360 GB/s HBM.
"""

import math
import sys

import numpy as np

sys.path.insert(0, "/opt/trn_rl_repo")

N_DIAGRAMS = 64
N_POINTS = 128
S = 64  # image is S x S
N_CORES = 8
N_PER_CORE = N_DIAGRAMS // N_CORES  # 8 diagrams per core
GRID_LO = np.float32(-3.0)
GRID_STEP = np.float32(6.0) / np.float32(S)

_BUILT = {}


def _build():
    """Build the single-core Bass program (SPMD: same program on all cores)."""
    if "nc" in _BUILT:
        return _BUILT["nc"]

    import concourse.bass as bass
    import concourse.mybir as mybir
    from concourse import bacc
    from concourse.tile import TileContext

    f32 = mybir.dt.float32
    AF = mybir.ActivationFunctionType
    OP = mybir.AluOpType

    nc = bacc.Bacc()

    # [p, k*8 + n]: cols 0..7 = d0 (birth x), cols 8..15 = raw y coords
    diag = nc.declare_dram_parameter("diag", [N_POINTS, 2 * N_PER_CORE], f32, isOutput=False)
    # [128, 129]: cols 0:64 X grid row, 64:128 Y grid row (identical values), col 128 variance
    grids = nc.declare_dram_parameter("grids", [128, 2 * S + 1], f32, isOutput=False)
    out = nc.declare_dram_parameter(
        "out", [N_PER_CORE * N_POINTS, S * S], f32, isOutput=True
    )

    with TileContext(nc) as tc:
        with (
            tc.tile_pool(name="const", bufs=1) as cpool,
            tc.tile_pool(name="big", bufs=3) as bigpool,
        ):
            gt = cpool.tile([128, 2 * S + 1], f32)
            nc.sync.dma_start(out=gt[:], in_=grids[:])
            dt = cpool.tile([N_POINTS, 2 * N_PER_CORE], f32)
            nc.sync.dma_start(out=dt[:], in_=diag[:])

            # --- scalar constants, per-partition [128,1] ---
            var = gt[:, 2 * S : 2 * S + 1]
            twov2 = cpool.tile([128, 1], f32)
            nc.vector.tensor_mul(twov2[:], var, var)
            nc.vector.tensor_scalar_mul(twov2[:], twov2[:], 2.0)
            cpos = cpool.tile([128, 1], f32)  # c = 1/(2 v^2)
            nc.vector.reciprocal(cpos[:], twov2[:])
            negc = cpool.tile([128, 1], f32)
            nc.vector.tensor_scalar_mul(negc[:], cpos[:], -1.0)
            amp = cpool.tile([128, 1], f32)  # A = 1/(2 pi v^2) = c/pi
            nc.vector.tensor_scalar_mul(amp[:], cpos[:], 1.0 / math.pi)

            # --- persistence coordinate d1 = y - x ---
            pers = cpool.tile([N_POINTS, N_PER_CORE], f32)
            nc.vector.tensor_sub(
                pers[:], dt[:, N_PER_CORE : 2 * N_PER_CORE], dt[:, 0:N_PER_CORE]
            )

            # --- factor tables gx, gy: [128, n*64 + {j,i}] ---
            def factor_table(coord_ap, grid_ap, tag):
                # dx[p, n, j] = coord[p, n] - grid[j]
                dx = cpool.tile([N_POINTS, N_PER_CORE * S], f32, tag=f"{tag}_dx")
                dx3 = dx[:].rearrange("p (n j) -> p n j", j=S)
                c3 = coord_ap.rearrange("p (n u) -> p n u", u=1)
                g3 = grid_ap.rearrange("p (u j) -> p u j", u=1)
                b0, b1 = bass.broadcast_tensor_aps(c3, g3)
                nc.vector.tensor_sub(dx3, b0, b1)
                # sq = (dx * -c) * dx
                sq = cpool.tile([N_POINTS, N_PER_CORE * S], f32, tag=f"{tag}_sq")
                nc.vector.scalar_tensor_tensor(
                    sq[:], dx[:], negc[:], dx[:], OP.mult, OP.mult
                )
                g = cpool.tile([N_POINTS, N_PER_CORE * S], f32, tag=tag)
                nc.scalar.activation(g[:], sq[:], AF.Exp)
                return g

            gx = factor_table(dt[:, 0:N_PER_CORE], gt[:, 0:S], "gx")
            gy = factor_table(pers[:], gt[:, S : 2 * S], "gy")
            # fold amplitude into gy: gy *= A (per-partition scalar)
            nc.vector.tensor_scalar_mul(gy[:], gy[:], amp[:])

            # --- expansion: out[p, i*64+j] = gy[p, n*64+i] * gx[p, n*64+j] ---
            for n in range(N_PER_CORE):
                ot = bigpool.tile([N_POINTS, S * S], f32)
                o3 = ot[:].rearrange("p (i j) -> p i j", j=S)
                gyv = gy[:, n * S : (n + 1) * S].rearrange("p (i u) -> p i u", u=1)
                gxv = gx[:, n * S : (n + 1) * S].rearrange("p (u j) -> p u j", u=1)
                a0, a1 = bass.broadcast_tensor_aps(gyv, gxv)
                nc.vector.tensor_mul(o3, a0, a1)
                nc.sync.dma_start(
                    out=out[n * N_POINTS : (n + 1) * N_POINTS, :], in_=ot[:]
                )

    nc.compile()
    _BUILT["nc"] = nc
    return nc


def _make_in_maps(diagrams, variance):
    xs = GRID_LO + GRID_STEP * np.arange(S, dtype=np.float32)  # exact fp32 grid
    grids = np.empty((128, 2 * S + 1), np.float32)
    grids[:, 0:S] = xs[None, :]
    grids[:, S : 2 * S] = xs[None, :]
    grids[:, 2 * S] = np.float32(variance)
    in_maps = []
    for c in range(N_CORES):
        sh = diagrams[c * N_PER_CORE : (c + 1) * N_PER_CORE]  # [8, 128, 2]
        diag = np.empty((N_POINTS, 2 * N_PER_CORE), np.float32)
        diag[:, 0:N_PER_CORE] = sh[:, :, 0].T
        diag[:, N_PER_CORE : 2 * N_PER_CORE] = sh[:, :, 1].T
        in_maps.append({"diag": np.ascontiguousarray(diag), "grids": grids})
    return in_maps


def _gather(results):
    outs = [
        results[c]["out"].reshape(N_PER_CORE, N_POINTS, S, S) for c in range(N_CORES)
    ]
    return np.concatenate(outs, axis=0)[..., None].astype(np.float32)


def run_traced(diagrams, variance):
    """Run with NTFF profiling; returns (output, exec_time_ns or None)."""
    from concourse.bass_utils import run_bass_kernel_spmd

    nc = _build()
    in_maps = _make_in_maps(np.asarray(diagrams, np.float32), variance)
    res = run_bass_kernel_spmd(nc, in_maps, list(range(N_CORES)), trace=True)
    return _gather(res.results), res.exec_time_ns


def kernel(diagrams, variance):
    from concourse.bass_utils import run_bass_kernel_spmd

    nc = _build()
    in_maps = _make_in_maps(np.asarray(diagrams, np.float32), variance)
    res = run_bass_kernel_spmd(nc, in_maps, list(range(N_CORES)))
    return _gather(res.results)


# revision 9
# speedup vs baseline: 1.1972x; 1.1972x over previous
"""Trainium2 Bass kernel for nn_GaussianPerslayPhi (Gaussian persistence image).

out[n, p, i, j] = exp(-((d0-X_j)^2 + (d1-Y_i)^2) / (2 v^2)) / (2 pi v^2)
with d0 = diagrams[n,p,0], d1 = diagrams[n,p,1] - diagrams[n,p,0],
X_j = Y_j = -3 + (6/64)*j, output shape (64, 128, 64, 64, 1) fp32.

Key structure: the Gaussian separates into gx[n,p,j] * gy[n,p,i], two tiny
(128,512) factor tables per core.  Each core (8 total, data-parallel over n)
computes the factors with ScalarE exp, then expands them with one broadcast
tensor_tensor multiply per diagram ([128, 4096] tile, step-0 access patterns)
and streams the 2 MiB tiles to HBM.  The kernel is output-write bound:
16 MiB/core at ~360 GB/s HBM.
"""

import math
import sys

import numpy as np

sys.path.insert(0, "/opt/trn_rl_repo")

N_DIAGRAMS = 64
N_POINTS = 128
S = 64  # image is S x S
N_CORES = 8
N_PER_CORE = N_DIAGRAMS // N_CORES  # 8 diagrams per core
GRID_LO = np.float32(-3.0)
GRID_STEP = np.float32(6.0) / np.float32(S)

_BUILT = {}


def _build():
    """Build the single-core Bass program (SPMD: same program on all cores)."""
    if "nc" in _BUILT:
        return _BUILT["nc"]

    import concourse.bass as bass
    import concourse.mybir as mybir
    from concourse import bacc
    from concourse.tile import TileContext

    f32 = mybir.dt.float32
    AF = mybir.ActivationFunctionType
    OP = mybir.AluOpType

    nc = bacc.Bacc()

    # [p, k*8 + n]: cols 0..7 = d0 (birth x), cols 8..15 = raw y coords
    diag = nc.declare_dram_parameter("diag", [N_POINTS, 2 * N_PER_CORE], f32, isOutput=False)
    # [128, 129]: cols 0:64 X grid row, 64:128 Y grid row (identical values), col 128 variance
    grids = nc.declare_dram_parameter("grids", [128, 2 * S + 1], f32, isOutput=False)
    out = nc.declare_dram_parameter(
        "out", [N_PER_CORE * N_POINTS, S * S], f32, isOutput=True
    )

    with TileContext(nc) as tc:
        with (
            tc.tile_pool(name="const", bufs=1) as cpool,
            tc.tile_pool(name="big", bufs=4) as bigpool,
        ):
            # dummy activation with no deps: schedules first on ACT, so the
            # exp table-set load (~2.7us) overlaps the input DMAs
            warm = cpool.tile([128, 1], f32)
            nc.gpsimd.memset(warm[:], 0.0)
            nc.scalar.activation(warm[:], warm[:], AF.Exp)

            gt = cpool.tile([128, 2 * S + 1], f32)
            nc.sync.dma_start(out=gt[:], in_=grids[:])
            dt = cpool.tile([N_POINTS, 2 * N_PER_CORE], f32)
            nc.sync.dma_start(out=dt[:], in_=diag[:])

            # --- scalar constants, per-partition [128,1] ---
            var = gt[:, 2 * S : 2 * S + 1]
            m2v2 = cpool.tile([128, 1], f32)
            nc.vector.tensor_mul(m2v2[:], var, var)
            nc.vector.tensor_scalar_mul(m2v2[:], m2v2[:], -2.0)  # -2 v^2
            negc = cpool.tile([128, 1], f32)  # -c = -1/(2 v^2)
            nc.vector.reciprocal(negc[:], m2v2[:])
            amp = cpool.tile([128, 1], f32)  # A = 1/(2 pi v^2) = -negc/pi
            nc.vector.tensor_scalar_mul(amp[:], negc[:], -1.0 / math.pi)

            # --- persistence coordinate d1 = y - x ---
            pers = cpool.tile([N_POINTS, N_PER_CORE], f32)
            nc.vector.tensor_sub(
                pers[:], dt[:, N_PER_CORE : 2 * N_PER_CORE], dt[:, 0:N_PER_CORE]
            )

            # --- factor tables gx, gy: [128, n*64 + {j,i}] ---
            def factor_table(coord_ap, grid_ap, tag):
                # dx[p, n, j] = coord[p, n] - grid[j]
                dx = cpool.tile([N_POINTS, N_PER_CORE * S], f32, tag=f"{tag}_dx")
                dx3 = dx[:].rearrange("p (n j) -> p n j", j=S)
                c3 = coord_ap.rearrange("p (n u) -> p n u", u=1)
                g3 = grid_ap.rearrange("p (u j) -> p u j", u=1)
                b0, b1 = bass.broadcast_tensor_aps(c3, g3)
                nc.vector.tensor_sub(dx3, b0, b1)
                # sq = (dx * -c) * dx
                sq = cpool.tile([N_POINTS, N_PER_CORE * S], f32, tag=f"{tag}_sq")
                nc.vector.scalar_tensor_tensor(
                    sq[:], dx[:], negc[:], dx[:], OP.mult, OP.mult
                )
                g = cpool.tile([N_POINTS, N_PER_CORE * S], f32, tag=tag)
                nc.scalar.activation(g[:], sq[:], AF.Exp)
                return g

            gx = factor_table(dt[:, 0:N_PER_CORE], gt[:, 0:S], "gx")
            gy = factor_table(pers[:], gt[:, S : 2 * S], "gy")
            # fold amplitude into gy: gy *= A (per-partition scalar)
            nc.vector.tensor_scalar_mul(gy[:], gy[:], amp[:])

            # --- expansion: out[p, i*64+j] = gy[p, n*64+i] * gx[p, n*64+j] ---
            # Each diagram's 2 MiB image is built in two 1 MiB halves (i in
            # [0,32) then [32,64)) and streamed out on alternating HWDGE
            # rings (SP / ACT) so the two rings' fixed costs overlap.
            H = S // 2
            for n in range(N_PER_CORE):
                for h in range(2):
                    ot = bigpool.tile([N_POINTS, H * S], f32, tag="ot")
                    o3 = ot[:].rearrange("p (i j) -> p i j", j=S)
                    gyv = gy[
                        :, n * S + h * H : n * S + (h + 1) * H
                    ].rearrange("p (i u) -> p i u", u=1)
                    gxv = gx[:, n * S : (n + 1) * S].rearrange(
                        "p (u j) -> p u j", u=1
                    )
                    a0, a1 = bass.broadcast_tensor_aps(gyv, gxv)
                    nc.vector.tensor_mul(o3, a0, a1)
                    eng = nc.sync if (n * 2 + h) % 2 == 0 else nc.scalar
                    eng.dma_start(
                        out=out[
                            n * N_POINTS : (n + 1) * N_POINTS,
                            h * H * S : (h + 1) * H * S,
                        ],
                        in_=ot[:],
                    )

    nc.compile()
    _BUILT["nc"] = nc
    return nc


def _make_in_maps(diagrams, variance):
    xs = GRID_LO + GRID_STEP * np.arange(S, dtype=np.float32)  # exact fp32 grid
    grids = np.empty((128, 2 * S + 1), np.float32)
    grids[:, 0:S] = xs[None, :]
    grids[:, S : 2 * S] = xs[None, :]
    grids[:, 2 * S] = np.float32(variance)
    in_maps = []
    for c in range(N_CORES):
        sh = diagrams[c * N_PER_CORE : (c + 1) * N_PER_CORE]  # [8, 128, 2]
        diag = np.empty((N_POINTS, 2 * N_PER_CORE), np.float32)
        diag[:, 0:N_PER_CORE] = sh[:, :, 0].T
        diag[:, N_PER_CORE : 2 * N_PER_CORE] = sh[:, :, 1].T
        in_maps.append({"diag": np.ascontiguousarray(diag), "grids": grids})
    return in_maps


def _gather(results):
    outs = [
        results[c]["out"].reshape(N_PER_CORE, N_POINTS, S, S) for c in range(N_CORES)
    ]
    return np.concatenate(outs, axis=0)[..., None].astype(np.float32)


def run_traced(diagrams, variance):
    """Run with NTFF profiling; returns (output, exec_time_ns or None)."""
    from concourse.bass_utils import run_bass_kernel_spmd

    nc = _build()
    in_maps = _make_in_maps(np.asarray(diagrams, np.float32), variance)
    res = run_bass_kernel_spmd(nc, in_maps, list(range(N_CORES)), trace=True)
    return _gather(res.results), res.exec_time_ns


def kernel(diagrams, variance):
    from concourse.bass_utils import run_bass_kernel_spmd

    nc = _build()
    in_maps = _make_in_maps(np.asarray(diagrams, np.float32), variance)
    res = run_bass_kernel_spmd(nc, in_maps, list(range(N_CORES)))
    return _gather(res.results)
